# revision 1
# baseline (speedup 1.0000x reference)
"""PillarQueryAndGroup kernel for Trainium2 (8 NeuronCores, SPMD).

reference semantics:
    gpf  = point_features[psi]            # (L, 64)
    gxyz = xyz[psi]                       # (L, 3)
    gctr = pillar_centers[pli]            # (L, 3)
    absl = gxyz + offset                  # offset = (0, -40, -3)
    rel  = gxyz - gctr
    out  = concat([gpf, absl, rel], 1)    # (L, 70)
    returns (pillar_indices, pli, out, ones(1))

Device strategy (per core, L sharded contiguously 8 ways):
  - host prep: table = [point_features | xyz + offset]  (N, 67) -> one gather
    yields cols 0:67 of the output directly (cols 64:67 are absl).
  - rel = absl - (pillar_centers + offset)[pli].  pli is sorted, so
    (centers+off)[pli] is piecewise-constant with <= M value changes.  The host
    ships sparse deltas E (dense (L,3), zero except at run switches, with exact
    re-bases every 64 pairs); the device reconstructs the expanded centers with
    a segmented log-doubling prefix scan (vector engine) and subtracts.
  - the (L, 64+3) gather runs as SWDGE indirect DMAs, 128 rows per call
    (one row per SBUF partition; HW consumes exactly one offset per partition).
  - output written as large contiguous per-partition DMAs.
"""
import sys
import numpy as np

sys.path.insert(0, "/opt/trn_rl_repo")

import concourse.bass as bass
import concourse.bacc as bacc
import concourse.tile as tile
from concourse import mybir
from concourse import bass_utils

# problem constants (hardcoded per harness contract)
N, C, M, L = 200000, 64, 50000, 2000000
OFFSET = np.array([0.0, -40.0, -3.0], np.float32)

NCORES = 8
P = 128
FSEG = 64                    # pairs per scan segment / gather tile column
TSEG = 31                    # segments per partition
Q = FSEG * TSEG              # 1984 pair slots per partition
LCORE = P * Q                # 253952 padded pairs per core
LPAD = NCORES * LCORE        # 2031616 >= L

_program_cache = {}


def _build_program():
    if "nc" in _program_cache:
        return _program_cache["nc"]
    nc = bacc.Bacc("TRN2", target_bir_lowering=False, debug=False, num_devices=NCORES)
    dt = mybir.dt
    table = nc.dram_tensor("table", [N, 67], dt.float32, kind="ExternalInput").ap()
    psi = nc.dram_tensor("psi", [P, Q], dt.int32, kind="ExternalInput").ap()
    ed = nc.dram_tensor("ed", [P, 3 * Q], dt.float32, kind="ExternalInput").ap()
    out = nc.dram_tensor("out", [P, Q * 70], dt.float32, kind="ExternalOutput").ap()

    with tile.TileContext(nc) as tc:
        with tc.tile_pool(name="persist", bufs=1) as pp, \
             tc.tile_pool(name="work", bufs=4) as wp:
            psi_s = pp.tile([P, Q], dt.int32)
            nc.sync.dma_start(out=psi_s[:, :], in_=psi[:, :])

            # segmented prefix scan of deltas -> expanded (centers + offset)
            ea = pp.tile([P, 3, TSEG, FSEG], dt.float32)
            eb = pp.tile([P, 3, TSEG, FSEG], dt.float32)
            nc.sync.dma_start(out=ea[:, :, :, :], in_=ed[:, :])
            src, dst = ea, eb
            for shift in (1, 2, 4, 8, 16, 32):
                nc.vector.tensor_tensor(
                    out=dst[:, :, :, shift:],
                    in0=src[:, :, :, shift:],
                    in1=src[:, :, :, : FSEG - shift],
                    op=mybir.AluOpType.add,
                )
                nc.vector.tensor_copy(out=dst[:, :, :, :shift], in_=src[:, :, :, :shift])
                src, dst = dst, src
            scan = src  # == ea after 6 passes

            for t in range(TSEG):
                ot = wp.tile([P, FSEG, 70], dt.float32, tag="ot")
                for g in range(FSEG):
                    j = t * FSEG + g
                    nc.gpsimd.indirect_dma_start(
                        out=ot[:, g, 0:67],
                        out_offset=None,
                        in_=table[:, :],
                        in_offset=bass.IndirectOffsetOnAxis(ap=psi_s[:, j:j + 1], axis=0),
                    )
                # rel = absl - expanded_centers
                nc.vector.tensor_tensor(
                    out=ot[:, :, 67:70],
                    in0=ot[:, :, 64:67],
                    in1=scan[:, :, t, :].transpose([0, 2, 1]),
                    op=mybir.AluOpType.subtract,
                )
                nc.sync.dma_start(out=out[:, t * FSEG * 70:(t + 1) * FSEG * 70],
                                  in_=ot[:, :, :])
    nc.compile()
    _program_cache["nc"] = nc
    return nc


def _host_prep(xyz, point_features, pillar_centers, point_set_indices, pillar_set_indices):
    table = np.concatenate([point_features, xyz + OFFSET[None, :]], axis=1)
    table = np.ascontiguousarray(table, dtype=np.float32)          # (N, 67)
    centers_adj = (pillar_centers + OFFSET[None, :]).astype(np.float32)

    psi = np.zeros(LPAD, np.int32)
    psi[:L] = point_set_indices
    pli = np.zeros(LPAD, np.int32)
    pli[:L] = pillar_set_indices
    pli[L:] = pillar_set_indices[-1]

    # dense deltas with exact re-bases at every segment (FSEG) boundary
    vals = centers_adj[pli]                                        # (LPAD, 3)
    e = np.zeros((LPAD, 3), np.float32)
    e[1:] = vals[1:] - vals[:-1]
    e[0] = vals[0]
    seg_starts = np.arange(0, LPAD, FSEG)
    e[seg_starts] = vals[seg_starts]

    in_maps = []
    for c in range(NCORES):
        lo = c * LCORE
        psi_c = psi[lo:lo + LCORE].reshape(P, Q)
        e_c = e[lo:lo + LCORE].reshape(P, TSEG, FSEG, 3)
        e_c = np.ascontiguousarray(e_c.transpose(0, 3, 1, 2)).reshape(P, 3 * Q)
        in_maps.append({"table": table, "psi": psi_c, "ed": e_c})
    return in_maps


def kernel(xyz, point_features, pillar_centers, pillar_indices,
           point_set_indices, pillar_set_indices):
    nc = _build_program()
    in_maps = _host_prep(xyz, point_features, pillar_centers,
                         point_set_indices, pillar_set_indices)
    res = bass_utils.run_bass_kernel_spmd(nc, in_maps, core_ids=list(range(NCORES)))
    parts = [res.results[c]["out"].reshape(LCORE, 70) for c in range(NCORES)]
    group_features = np.concatenate(parts, axis=0)[:L]
    indice2bev = np.ones((1,), np.float32)
    return pillar_indices, pillar_set_indices, group_features, indice2bev


# revision 2
# speedup vs baseline: 1.0168x; 1.0168x over previous
"""PillarQueryAndGroup kernel for Trainium2 (8 NeuronCores, SPMD).

Reference semantics:
    gpf  = point_features[psi]            # (L, 64)
    gxyz = xyz[psi]                       # (L, 3)
    gctr = pillar_centers[pli]            # (L, 3)
    absl = gxyz + offset                  # offset = (0, -40, -3)
    rel  = gxyz - gctr
    out  = concat([gpf, absl, rel], 1)    # (L, 70)
    returns (pillar_indices, pli, out, ones(1))

Strategy:
  * Pairs are sharded to the core that owns their point's index window
    (core c owns point rows [c*25000, (c+1)*25000)) — "shard points, keep
    pairs local to their point shard".  Local point indices then fit int16,
    which unlocks the bulk descriptor-gather instruction (InstDMAGatherAnt):
    1024 gathered rows per instruction instead of 128 rows per SWDGE
    indirect DMA, removing the per-instruction descriptor-generation
    bottleneck.
  * The host packs a padded table  [point_features | xyz+offset | pad]
    with 512 B row stride (256 B-multiple, required by the descriptor
    stride field); each gather pulls a 280 B payload (70 f32) straight into
    output-shaped SBUF tiles, so gathered rows need no re-copying.
  * rel needs (pillar_centers+offset)[pli].  pli is sorted, so that gather
    is piecewise-constant with at most M changes; any core's pair
    subsequence stays sorted.  The host ships dense sparse-deltas E (zero
    except at run switches, exact re-bases every 64 slots) and the device
    reconstructs the expanded centers with a segmented log-doubling prefix
    scan on the vector engine, then subtracts in place over the 3 pad
    columns.  No per-pair center gather at all.
  * Output is written with large contiguous per-partition DMAs; the host
    scatters each core's compact output back to original pair order.
"""
import sys
import numpy as np

sys.path.insert(0, "/opt/trn_rl_repo")

import concourse.bacc as bacc
import concourse.tile as tile
from concourse import mybir
from concourse import ap_utils
from concourse import bass_utils

N, C, M, L = 200000, 64, 50000, 2000000
OFFSET = np.array([0.0, -40.0, -3.0], np.float32)

NCORES = 8
P = 128
W = N // NCORES              # 25000 point rows per core window
NIDX = 1024                  # rows per dma_gather call (desc-ring/scratch limit)
SC = NIDX // P               # 8 slots per partition per call
KC = 8                       # calls per tile
FSEG = SC * KC               # 64 slots per partition per tile == scan segment
TILE_PAIRS = NIDX * KC       # 8192

_cache = {}


def _dma_gather_raw(gp, out_ap, in_ap, idxs_ap, num_idxs, elem_size, elem_step):
    """InstDMAGatherAnt lowering with elem_size decoupled from the row stride.

    bass.dma_gather asserts elem_size_bytes % 256 == 0, but per the ucode that
    restriction is transpose-only; non-transpose needs only the row *stride*
    to be a 256 B multiple (descriptor stride field is in 256 B units) and the
    payload 8 B-aligned.  gathered[p, s, :] = in[idxs[s*128+p], :elem_size].
    """
    nc = gp.bass
    assert idxs_ap.dtype == mybir.dt.int16
    dts = mybir.dt.size(in_ap.dtype)
    stride_bytes = elem_step * dts
    assert stride_bytes % 256 == 0 and stride_bytes // 256 < 256
    assert (elem_size * dts) % 8 == 0
    assert in_ap.ap[0][0] == elem_step and in_ap.ap[-1][1] == elem_size
    assert ap_utils.ap_is_contiguous(out_ap.ap[1:])
    assert ap_utils.ap_is_contiguous(idxs_ap.ap[1:])
    assert out_ap.ap[0][1] * out_ap.ap[1][1] == ((num_idxs + 127) // 128) * 128
    assert out_ap.ap[-1][1] == elem_size

    _in_ap = gp.lower_ap_dma(in_ap, for_custom_bir_dma=True)
    return gp.add_instruction(
        mybir.InstDMAGatherAnt(
            name=nc.get_next_instruction_name(),
            ins=[*_in_ap, gp.lower_ap(idxs_ap),
                 gp.lower_val_access(gp.to_reg(num_idxs))],
            outs=[gp.lower_ap(out_ap)],
            transpose=False,
            num_idxs=num_idxs,
            elem_size=elem_size,
            stride_bytes_256=stride_bytes // 256,
            gen_mode=0,
            single_packet=True,
            queue_num=0,
            sbuf_tokens_per_rank=0,
            sbuf_free_dim_per_rank=0,
            sbuf_free_dim_pad_per_rank=0,
            sbuf_byte_offset=0,
        )
    )


def _build_program(ntiles):
    key = ("nc", ntiles)
    if key in _cache:
        return _cache[key]
    nc = bacc.Bacc("TRN2", target_bir_lowering=False, debug=False, num_devices=NCORES)
    dt = mybir.dt
    ncall = ntiles * KC
    q = ntiles * FSEG                       # slots per partition
    table = nc.dram_tensor("table", [W, 128], dt.float32, kind="ExternalInput").ap()
    idx = nc.dram_tensor("idx", [P, ncall * 64], dt.int16, kind="ExternalInput").ap()
    ed = nc.dram_tensor("ed", [P, 3 * q], dt.float32, kind="ExternalInput").ap()
    out = nc.dram_tensor("out", [P, q * 70], dt.float32, kind="ExternalOutput").ap()

    with tile.TileContext(nc) as tc:
        with tc.tile_pool(name="persist", bufs=1) as pp, \
             tc.tile_pool(name="work", bufs=4) as wp:
            idx_s = pp.tile([P, ncall * 64], dt.int16)
            nc.sync.dma_start(out=idx_s[:, :], in_=idx[:, :])

            # segmented log-doubling prefix scan of center deltas
            ea = pp.tile([P, 3, ntiles, FSEG], dt.float32)
            eb = pp.tile([P, 3, ntiles, FSEG], dt.float32)
            nc.sync.dma_start(out=ea[:, :, :, :], in_=ed[:, :])
            src, dst = ea, eb
            for shift in (1, 2, 4, 8, 16, 32):
                nc.vector.tensor_tensor(
                    out=dst[:, :, :, shift:], in0=src[:, :, :, shift:],
                    in1=src[:, :, :, : FSEG - shift], op=mybir.AluOpType.add)
                nc.vector.tensor_copy(out=dst[:, :, :, :shift], in_=src[:, :, :, :shift])
                src, dst = dst, src
            scan = src

            for tt in range(ntiles):
                ot = wp.tile([P, FSEG, 70], dt.float32, tag="ot")
                for k in range(KC):
                    t = tt * KC + k
                    _dma_gather_raw(
                        nc.gpsimd,
                        out_ap=ot[:, k * SC:(k + 1) * SC, :],
                        in_ap=table[:, 0:70],
                        idxs_ap=idx_s[:, t * 64:(t + 1) * 64],
                        num_idxs=NIDX,
                        elem_size=70,
                        elem_step=128,
                    )
                nc.vector.tensor_tensor(
                    out=ot[:, :, 67:70], in0=ot[:, :, 64:67],
                    in1=scan[:, :, tt, :].transpose([0, 2, 1]),
                    op=mybir.AluOpType.subtract)
                nc.sync.dma_start(out=out[:, tt * FSEG * 70:(tt + 1) * FSEG * 70],
                                  in_=ot[:, :, :])
    nc.compile()
    _cache[key] = nc
    return nc


def _wrap16(lists):
    """(ncall, 1024) int16 -> (128, ncall*64): list position i lives at
    partition i%16, column i//16; replicated across the 8 partition groups."""
    ncall = lists.shape[0]
    w = lists.reshape(ncall, 64, 16).transpose(2, 0, 1).reshape(16, ncall * 64)
    return np.tile(w, (8, 1))


def kernel(xyz, point_features, pillar_centers, pillar_indices,
           point_set_indices, pillar_set_indices):
    xyz = np.ascontiguousarray(np.asarray(xyz, np.float32))
    point_features = np.ascontiguousarray(np.asarray(point_features, np.float32))
    pillar_centers = np.ascontiguousarray(np.asarray(pillar_centers, np.float32))
    psi = np.asarray(point_set_indices, np.int32)
    pli = np.asarray(pillar_set_indices, np.int32)

    table = np.zeros((N, 128), np.float32)
    table[:, :C] = point_features
    table[:, C:C + 3] = xyz + OFFSET[None, :]
    centers_adj = pillar_centers + OFFSET[None, :]

    win = psi // W
    sels = [np.flatnonzero(win == c) for c in range(NCORES)]
    max_count = max(len(s) for s in sels)
    ntiles = max(1, -(-max_count // TILE_PAIRS))
    lcap = ntiles * TILE_PAIRS
    ncall = ntiles * KC
    q = ntiles * FSEG

    nc = _build_program(ntiles)

    in_maps = []
    for c in range(NCORES):
        sel = sels[c]
        cnt = len(sel)
        local = np.zeros(lcap, np.int16)
        local[:cnt] = (psi[sel] - c * W).astype(np.int16)
        idx_np = _wrap16(local.reshape(ncall, NIDX))

        pli_c = np.zeros(lcap, np.int32)
        if cnt:
            pli_c[:cnt] = pli[sel]
            pli_c[cnt:] = pli[sel][-1]
        vals = centers_adj[pli_c]                                   # (lcap, 3)
        # device pair d = t*1024 + s*128 + p sits at [partition p, slot j=t*8+s]
        vals_dev = vals.reshape(ncall, SC, P, 3).transpose(2, 0, 1, 3).reshape(P, q, 3)
        e = np.empty((P, q, 3), np.float32)
        e[:, 1:] = vals_dev[:, 1:] - vals_dev[:, :-1]
        e[:, 0] = vals_dev[:, 0]
        seg0 = np.arange(0, q, FSEG)
        e[:, seg0] = vals_dev[:, seg0]
        ed = np.ascontiguousarray(
            e.reshape(P, ntiles, FSEG, 3).transpose(0, 3, 1, 2)).reshape(P, 3 * q)

        in_maps.append({
            "table": table[c * W:(c + 1) * W],
            "idx": idx_np,
            "ed": ed,
        })

    res = bass_utils.run_bass_kernel_spmd(nc, in_maps, core_ids=list(range(NCORES)))

    group_features = np.empty((L, 70), np.float32)
    for c in range(NCORES):
        sel = sels[c]
        o = res.results[c]["out"].reshape(P, ncall, SC, 70).transpose(1, 2, 0, 3)
        group_features[sel] = o.reshape(lcap, 70)[:len(sel)]
    indice2bev = np.ones((1,), np.float32)
    return (np.asarray(pillar_indices, np.int32), pli, group_features, indice2bev)
